# revision 14
# baseline (speedup 1.0000x reference)
"""Trainium2 Bass kernel for LlamaMultiheadLatentAttention (v2).

Contract: kernel(**inputs) takes FULL fp32 inputs (as produced by
reference.setup_inputs) and returns the FULL fp32 output [2, 1024, 4096].

Sharding (8 cores, no collectives): core c handles batch b = c//4 and
head-group g = c%4 (8 query heads, 2 kv heads, 8 latent heads). q/k/v and
latent projections are column-sharded per head-group; o_proj/latent_o_proj
are row-sharded, so each core emits a partial output sum and the host adds
the 4 partials per batch.

v2 changes vs baseline (1005us):
  - lk folded on host: lk_eff = w_lq @ w_lk, so the replicated lq
    projection (and its 128 extra matmuls/core) disappears.
  - attention: exp activations run on [128,1024] score PAIRS (halves the
    per-call 352-cycle ScalarE overhead); softmax denominators come from a
    bf16 tree-sum on VectorE + ONE ones-matmul per (head, i-block) instead
    of 12 accumulated ones-matmuls; the M=1 denominator matmuls of 4
    consecutive iterations are col-tiled to partitions 0/32/64/96 of one
    PSUM bank so a single DVE reciprocal (8 cyc/elem, partition-parallel)
    serves 4 iterations; attention output is copied out UNNORMALIZED
    (frees the PSUM bank immediately -> no TensorE stall on the
    reciprocal chain) and rescaled in place afterwards.
  - phase D weights prefetched during attention; B-phase DMAs overlapped
    with compute (8-chunk xt, early wv/wlv issue, k-proj between the two
    wlv half-loads).
"""

import numpy as np
import ml_dtypes

import concourse.bass as bass
import concourse.mybir as mybir
import concourse.tile as tile
from concourse import bacc
from concourse.bass_utils import run_bass_kernel_spmd

BF16 = ml_dtypes.bfloat16

B, S, D = 2, 1024, 4096
H, KVH, HD = 32, 8, 128
GROUPS = H // KVH
LAT, LH = 1024, 32
THETA = 10000.0
SCALE = 1.0 / float(np.sqrt(HD))

NCORES = 8
TP = 4                 # head-group shards
HL = H // TP           # 8 local q heads
KVL = KVH // TP        # 2 local kv heads
LHL = LH // TP         # 8 local latent heads

f32 = mybir.dt.float32
bf16 = mybir.dt.bfloat16

D_T = D // 128         # 32 k-tiles over model dim
S_T = S // 128         # 8 token tiles of 128
IB = 2                 # i-blocks of 512 in attention


def _build_program():
    nc = bacc.Bacc("TRN2", target_bir_lowering=False, debug=False)

    xt_d = nc.dram_tensor("xt", [128, D_T, S], bf16, kind="ExternalInput")
    wq_d = nc.dram_tensor("wq", [HL, 128, D_T, 128], bf16, kind="ExternalInput")
    wk_d = nc.dram_tensor("wk", [KVL, 128, D_T, 128], bf16, kind="ExternalInput")
    wlk_d = nc.dram_tensor("wlk", [LHL, 128, D_T, 128], bf16,
                           kind="ExternalInput")
    wv_d = nc.dram_tensor("wv", [128, D_T, KVL * HD], bf16, kind="ExternalInput")
    wlv_d = nc.dram_tensor("wlv", [2, 128, D_T, 512], bf16,
                           kind="ExternalInput")
    wo_d = nc.dram_tensor("wo", [8, 128, HL, 512], bf16, kind="ExternalInput")
    wlo_d = nc.dram_tensor("wlo", [8, 128, LHL, 512], bf16, kind="ExternalInput")
    cos_d = nc.dram_tensor("cosT", [HD, S], f32, kind="ExternalInput")
    sin_d = nc.dram_tensor("sinTs", [HD, S], f32, kind="ExternalInput")
    mask_d = nc.dram_tensor("maskP2", [128, 2, 1024], bf16, kind="ExternalInput")
    out_d = nc.dram_tensor("out", [S, D], f32, kind="ExternalOutput")

    out_ap = out_d.ap().rearrange("(tt p) d -> p tt d", p=128)

    with tile.TileContext(nc) as tc:
        with tc.tile_pool(name="const", bufs=1) as constp, \
             tc.tile_pool(name="acts", bufs=1) as acts:

            cosT = constp.tile([HD, S], f32, tag="cosT")
            sinTs = constp.tile([HD, S], f32, tag="sinTs")
            maskP2 = constp.tile([128, 2, 1024], bf16, tag="maskP2")
            ones = constp.tile([128, 1], bf16, tag="ones")
            nc.sync.dma_start(cosT[:], cos_d.ap())
            nc.sync.dma_start(sinTs[:], sin_d.ap())
            nc.sync.dma_start(maskP2[:], mask_d.ap())
            nc.vector.memset(ones[:], 1.0)

            # persistent activations (bf16)
            qT = acts.tile([128, HL, S], bf16, tag="qT")
            kT = acts.tile([128, KVL, S], bf16, tag="kT")
            lkT = acts.tile([128, LHL, S], bf16, tag="lkT")
            v_sb = acts.tile([128, S_T, KVL * HD], bf16, tag="v")
            lv_sb = acts.tile([128, S_T, LHL * HD], bf16, tag="lv")

            with tc.tile_pool(name="xt", bufs=1) as xtp, \
                 tc.tile_pool(name="wvlv", bufs=1) as wvp, \
                 tc.tile_pool(name="wstr", bufs=2) as wstr, \
                 tc.tile_pool(name="rope", bufs=2) as ropep, \
                 tc.tile_pool(name="ps_b", bufs=1, space="PSUM") as psb:

                xt = xtp.tile([128, D_T, S], bf16, tag="xt")
                for c in range(8):
                    nc.sync.dma_start(
                        xt[:, bass.ts(c, D_T // 8), :],
                        xt_d.ap()[:, bass.ts(c, D_T // 8), :])

                def rope_to(dst, ps, ib):
                    sl = bass.ts(ib, 512)
                    rt = ropep.tile([128, 512], f32, tag="rt")
                    qc = ropep.tile([128, 512], f32, tag="qc")
                    nc.vector.tensor_tensor(
                        rt[0:64, :], ps[64:128, :], sinTs[0:64, sl],
                        mybir.AluOpType.mult)
                    nc.vector.tensor_tensor(
                        rt[64:128, :], ps[0:64, :], sinTs[64:128, sl],
                        mybir.AluOpType.mult)
                    nc.vector.tensor_tensor(
                        qc[:], ps[:], cosT[:, sl], mybir.AluOpType.mult)
                    nc.vector.tensor_add(dst, qc[:], rt[:])

                def proj_fm(w_dram, n_tiles, dst):
                    # dst[:, nt, :] = rope(w[:, nt-block].T @ xt)
                    for nt in range(n_tiles):
                        wt = wstr.tile([128, D_T, 128], bf16, tag="w_fm")
                        nc.sync.dma_start(wt[:], w_dram.ap()[nt])
                        ps = [psb.tile([128, 512], f32, tag="ps_b1",
                                       bufs=4, name=f"ps_b1_{nt}_{ib}")
                              for ib in range(IB)]
                        for kt in range(D_T):
                            for ib in range(IB):
                                nc.tensor.matmul(
                                    ps[ib][:], wt[:, kt, :],
                                    xt[:, kt, bass.ts(ib, 512)],
                                    start=(kt == 0), stop=(kt == D_T - 1))
                        for ib in range(IB):
                            rope_to(dst[:, nt, bass.ts(ib, 512)], ps[ib][:], ib)

                # ---- q projection first (needs only wq[0] + xt to start) --
                proj_fm(wq_d, HL, qT)

                # ---- B2 weights (prefetch wlv half 0 + wv during q-proj) --
                wv_sb = wvp.tile([128, D_T, KVL * HD], bf16, tag="wv")
                for c in range(4):
                    nc.sync.dma_start(
                        wv_sb[:, bass.ts(c, D_T // 4), :],
                        wv_d.ap()[:, bass.ts(c, D_T // 4), :])

                def b2_half(half, wlv_sb):
                    hs = bass.ts(half, 512)
                    for tt in range(S_T):
                        ps_lv = psb.tile([128, 512], f32, tag="ps_lv",
                                         bufs=2, name=f"ps_lv_{half}_{tt}")
                        if half == 0:
                            ps_v = psb.tile([128, KVL * HD], f32, tag="ps_v",
                                            bufs=2, name=f"ps_v_{tt}")
                        for kt in range(D_T):
                            st = kt == 0
                            sp = kt == D_T - 1
                            lhs = xt[:, kt, bass.ts(tt, 128)]
                            nc.tensor.matmul(ps_lv[:], lhs, wlv_sb[:, kt, :],
                                             start=st, stop=sp)
                            if half == 0:
                                nc.tensor.matmul(ps_v[:], lhs, wv_sb[:, kt, :],
                                                 start=st, stop=sp)
                        nc.any.tensor_copy(lv_sb[:, tt, hs], ps_lv[:])
                        if half == 0:
                            nc.any.tensor_copy(v_sb[:, tt, :], ps_v[:])

                wlv0 = wvp.tile([128, D_T, 512], bf16, tag="wlvh",
                                name="wlvh_0")
                for c in range(2):
                    nc.sync.dma_start(
                        wlv0[:, bass.ts(c, D_T // 2), :],
                        wlv_d.ap()[0][:, bass.ts(c, D_T // 2), :])
                b2_half(0, wlv0)

                # wlv half 1 DMA hides behind k projection
                wlv1 = wvp.tile([128, D_T, 512], bf16, tag="wlvh",
                                name="wlvh_1")
                for c in range(2):
                    nc.sync.dma_start(
                        wlv1[:, bass.ts(c, D_T // 2), :],
                        wlv_d.ap()[1][:, bass.ts(c, D_T // 2), :])
                proj_fm(wk_d, KVL, kT)
                b2_half(1, wlv1)

                # latent k via host-folded weight (same shape as q proj)
                proj_fm(wlk_d, LHL, lkT)

            # ---- phase C: attention (+ phase D weight prefetch) ----------
            with tc.tile_pool(name="attnlat", bufs=1) as alp, \
                 tc.tile_pool(name="wop", bufs=2) as wop, \
                 tc.tile_pool(name="ost", bufs=4) as ost:
                attnT = alp.tile([128, HL, S], bf16, tag="attnT")
                latT = alp.tile([128, LHL, S], bf16, tag="latT")

                def load_wo(np_):
                    wo2 = wop.tile([128, HL, 1024], bf16, tag="wo2",
                                   name=f"wo2_{np_}")
                    wlo2 = wop.tile([128, LHL, 1024], bf16, tag="wlo2",
                                    name=f"wlo2_{np_}")
                    for u in range(2):
                        nc.sync.dma_start(wo2[:, :, bass.ts(u, 512)],
                                          wo_d.ap()[2 * np_ + u])
                        nc.sync.dma_start(wlo2[:, :, bass.ts(u, 512)],
                                          wlo_d.ap()[2 * np_ + u])
                    return wo2, wlo2

                wo_tiles = {0: load_wo(0), 1: load_wo(1)}

                with tc.tile_pool(name="pp", bufs=6) as pp, \
                     tc.tile_pool(name="dn", bufs=1) as dn, \
                     tc.tile_pool(name="ps_c", bufs=1, space="PSUM") as psc:

                    pending = []   # (dst_slice, slot) awaiting normalization
                    flush_q = []   # deferred normalization groups
                    dbatch = None
                    it = 0

                    def emit_flush():
                        # normalize a completed 4-iteration group, staged
                        # across two later iterations so no single VectorE
                        # block (esp. the 8-cyc/elem reciprocal) sits in
                        # front of the next iteration's mask/tree work
                        st = flush_q[0]
                        grp_dbatch, grp, stage, rec, recbs = st
                        if stage == 0:
                            rec = dn.tile([128, 512], f32, tag="rec", bufs=2)
                            nc.vector.reciprocal(rec[0:97, 0:256],
                                                 grp_dbatch[0:97, 0:256])
                            st[2] = 1
                            st[3] = rec
                            return
                        if stage == 1:
                            nc.vector.reciprocal(rec[0:97, 256:512],
                                                 grp_dbatch[0:97, 256:512])
                            for _, pslot in grp:
                                # partition_broadcast only reads partition 0
                                # on HW: stage the row there via SBUF DMA
                                if pslot == 0:
                                    src = rec[0:1, :]
                                else:
                                    r0 = dn.tile([1, 512], f32, tag="r0",
                                                 bufs=2)
                                    nc.sync.dma_start(
                                        r0[0:1, :],
                                        rec[32 * pslot:32 * pslot + 1, :])
                                    src = r0[0:1, :]
                                recb = dn.tile([128, 512], f32, tag="recb",
                                               bufs=4)
                                nc.gpsimd.partition_broadcast(recb[:], src)
                                recbs.append(recb)
                            st[2] = 2
                            return
                        for (pdsl, _), recb in zip(grp, recbs):
                            # GpSimd: VectorE is the C-phase bottleneck
                            nc.gpsimd.tensor_mul(pdsl, pdsl, recb[:])
                        flush_q.pop(0)

                    for vh in range(HL + LHL):
                        if vh < HL:
                            h = vh
                            ksrc = kT[:, h // GROUPS, :]
                            dst = attnT
                        else:
                            h = vh - HL
                            ksrc = lkT[:, h, :]
                            dst = latT
                        qsrc = qT[:, h, :]

                        for ib in range(IB):
                            if flush_q:
                                emit_flush()
                            npair = 2 * (ib + 1)
                            isl = bass.ts(ib, 512)
                            pts = []
                            for pb in range(npair):
                                ps_pair = psc.tile([128, 1024], f32,
                                                   tag="ps_pair", bufs=2)
                                for u in range(2):
                                    nc.tensor.matmul(
                                        ps_pair[:, bass.ts(u, 512)],
                                        ksrc[:, bass.ts(2 * pb + u, 128)],
                                        qsrc[:, isl], start=True, stop=True)
                                pt = pp.tile([128, 1024], bf16, tag="pt")
                                nc.scalar.activation(
                                    pt[:], ps_pair[:],
                                    mybir.ActivationFunctionType.Exp,
                                    scale=SCALE)
                                # diagonal pairs get the causal 0/1 mask
                                r = 2 * pb - 4 * ib
                                if r >= 0:
                                    nc.vector.tensor_tensor(
                                        pt[:], pt[:], maskP2[:, r // 2, :],
                                        mybir.AluOpType.mult)
                                pts.append(pt)

                            ps_o = psc.tile([128, 512], f32, tag="ps_o",
                                            bufs=2)
                            for jb in range(2 * npair):
                                if vh < HL:
                                    vsl = v_sb[:, jb, bass.ts(h // GROUPS, HD)]
                                else:
                                    vsl = lv_sb[:, jb, bass.ts(h, HD)]
                                nc.tensor.matmul(
                                    ps_o[:], vsl,
                                    pts[jb // 2][:, bass.ts(jb % 2, 512)],
                                    start=(jb == 0), stop=(jb == 2 * npair - 1))

                            # denominator: per-pair half-sum on VectorE, then
                            # accumulated M=1 ones-matmuls (TensorE has slack
                            # in this phase) col-tiled into partition 32*slot
                            psums = []
                            for pt in pts:
                                sp_ = dn.tile([128, 512], bf16, tag="tadd",
                                              bufs=6)
                                nc.vector.tensor_add(sp_[:], pt[:, 0:512],
                                                     pt[:, 512:1024])
                                psums.append(sp_)

                            slot = it % 4
                            if slot == 0:
                                dbatch = psc.tile([128, 512], f32,
                                                  tag="dbatch", bufs=2)
                                nc.vector.memset(dbatch[:], 1.0)
                            for pb, sp_ in enumerate(psums):
                                nc.tensor.matmul(
                                    dbatch[32 * slot:32 * slot + 1, :],
                                    ones[:, 0:1], sp_[:],
                                    start=(pb == 0),
                                    stop=(pb == len(psums) - 1),
                                    tile_position=(0, 32 * slot))

                            # unnormalized copy-out frees the PSUM bank now
                            dsl = dst[:, h, isl]
                            nc.any.tensor_copy(dsl, ps_o[:])
                            pending.append((dsl, slot))

                            if slot == 3:
                                flush_q.append([dbatch, pending, 0, None, []])
                                pending = []
                            it += 1

                    while flush_q:
                        emit_flush()

                # ---- phase D: output projections (row-sharded partials) --
                with tc.tile_pool(name="ps_f", bufs=4, space="PSUM") as psf:
                    for np_ in range(4):
                        wo2, wlo2 = wo_tiles.pop(np_)
                        if np_ + 2 < 4:
                            wo_tiles[np_ + 2] = load_wo(np_ + 2)
                        for tt in range(S_T):
                            ps0 = psf.tile([128, 512], f32, tag="ps_f")
                            ps1 = psf.tile([128, 512], f32, tag="ps_f")
                            for h in range(HL):
                                lhs = attnT[:, h, bass.ts(tt, 128)]
                                nc.tensor.matmul(ps0[:], lhs,
                                                 wo2[:, h, 0:512],
                                                 start=(h == 0), stop=False)
                                nc.tensor.matmul(ps1[:], lhs,
                                                 wo2[:, h, 512:1024],
                                                 start=(h == 0), stop=False)
                            for h in range(LHL):
                                lhs = latT[:, h, bass.ts(tt, 128)]
                                nc.tensor.matmul(ps0[:], lhs,
                                                 wlo2[:, h, 0:512],
                                                 start=False, stop=(h == LHL - 1))
                                nc.tensor.matmul(ps1[:], lhs,
                                                 wlo2[:, h, 512:1024],
                                                 start=False, stop=(h == LHL - 1))
                            for u, ps in enumerate((ps0, ps1)):
                                ot = ost.tile([128, 512], f32, tag="ot")
                                nc.any.tensor_copy(ot[:], ps[:])
                                nc.sync.dma_start(
                                    out_ap[:, tt, bass.ds(
                                        (2 * np_ + u) * 512, 512)],
                                    ot[:])

    nc.compile()
    return nc


_NC = None


def _get_program():
    global _NC
    if _NC is None:
        _NC = _build_program()
    return _NC


def _rope_tables():
    inv_freq = 1.0 / (THETA ** (np.arange(0, HD, 2, dtype=np.float32) / HD))
    t = np.arange(S, dtype=np.float32)
    freqs = np.outer(t, inv_freq)                       # [S, 64]
    emb = np.concatenate([freqs, freqs], axis=-1)       # [S, HD]
    cosT = np.cos(emb).T.astype(np.float32).copy()      # [HD, S]
    sinT = np.sin(emb).T.astype(np.float32)
    sinTs = np.concatenate([-sinT[:HD // 2], sinT[HD // 2:]], 0).astype(
        np.float32).copy()
    return cosT, sinTs


def _mask_patterns():
    # maskP2[p, t, u*512 + i] = 1.0 iff ((2t+u)*128 + p) <= i, i in [0,512)
    p = np.arange(128)[:, None, None]
    ju = np.arange(4)[None, :, None]        # jb index within diagonal block
    i = np.arange(512)[None, None, :]
    m = ((ju * 128 + p) <= i).astype(BF16)  # [128, 4, 512]
    return np.ascontiguousarray(
        m.reshape(128, 2, 2, 512).reshape(128, 2, 1024))


def _tile_w_fm(w, n_tiles, kt):
    # [K, n_tiles*128] -> [n_tiles, 128(p of K), kt, 128]
    K, N = w.shape
    assert K == kt * 128 and N == n_tiles * 128
    return np.ascontiguousarray(
        w.reshape(kt, 128, n_tiles, 128).transpose(2, 1, 0, 3)).astype(BF16)


def _tile_w_tm(w, kt):
    # [K, N] -> [128(p of K), kt, N]
    K, N = w.shape
    assert K == kt * 128
    return np.ascontiguousarray(
        w.reshape(kt, 128, N).transpose(1, 0, 2)).astype(BF16)


def _tile_w_lv(w):
    # [D, 1024] -> [2(half), 128(p of K), D_T, 512]
    t = _tile_w_tm(w, D_T)                  # [128, D_T, 1024]
    return np.ascontiguousarray(
        t.reshape(128, D_T, 2, 512).transpose(2, 0, 1, 3))


def _tile_w_out(w):
    # [1024, D] -> [8(nb), 128(p of rows), 8(h), 512]
    return np.ascontiguousarray(
        w.reshape(8, 128, D // 512, 512).transpose(2, 1, 0, 3)).astype(BF16)


def _make_in_maps(hidden_states, w_q, w_k, w_v, w_o, w_lq, w_lk, w_lv, w_lo):
    cosT, sinTs = _rope_tables()
    maskP2 = _mask_patterns()

    # host fold: lk_eff = w_lq @ w_lk  [D, LH*HD]
    lk_eff = np.asarray(w_lq, dtype=np.float32) @ np.asarray(
        w_lk, dtype=np.float32)

    g_maps = []
    for g in range(TP):
        qs = slice(g * HL * HD, (g + 1) * HL * HD)
        kvs = slice(g * KVL * HD, (g + 1) * KVL * HD)
        ls = slice(g * LHL * HD, (g + 1) * LHL * HD)
        g_maps.append({
            "wq": _tile_w_fm(np.asarray(w_q)[:, qs], HL, D_T),
            "wk": _tile_w_fm(np.asarray(w_k)[:, kvs], KVL, D_T),
            "wlk": _tile_w_fm(lk_eff[:, ls], LHL, D_T),
            "wv": _tile_w_tm(np.asarray(w_v)[:, kvs], D_T),
            "wlv": _tile_w_lv(np.asarray(w_lv)[:, ls]),
            "wo": _tile_w_out(np.asarray(w_o)[qs, :]),
            "wlo": _tile_w_out(np.asarray(w_lo)[ls, :]),
            "cosT": cosT,
            "sinTs": sinTs,
            "maskP2": maskP2,
        })

    xts = []
    for b in range(B):
        x = np.asarray(hidden_states[b], dtype=np.float32)       # [S, D]
        xts.append(np.ascontiguousarray(
            x.T.reshape(D_T, 128, S).transpose(1, 0, 2)).astype(BF16))

    in_maps = []
    for c in range(NCORES):
        b, g = divmod(c, TP)
        m = dict(g_maps[g])
        m["xt"] = xts[b]
        in_maps.append(m)
    return in_maps


def kernel(hidden_states, w_q, w_k, w_v, w_o, w_lq, w_lk, w_lv, w_lo):
    nc = _get_program()
    in_maps = _make_in_maps(hidden_states, w_q, w_k, w_v, w_o,
                            w_lq, w_lk, w_lv, w_lo)
    res = run_bass_kernel_spmd(nc, in_maps, list(range(NCORES))).results

    out = np.zeros((B, S, D), dtype=np.float32)
    for c in range(NCORES):
        b = c // TP
        out[b] += res[c]["out"]
    return out


# revision 18
# speedup vs baseline: 1.4205x; 1.4205x over previous
"""Trainium2 Bass kernel for LlamaMultiheadLatentAttention (v2).

Contract: kernel(**inputs) takes FULL fp32 inputs (as produced by
reference.setup_inputs) and returns the FULL fp32 output [2, 1024, 4096].

Sharding (8 cores, no collectives): core c handles batch b = c//4 and
head-group g = c%4 (8 query heads, 2 kv heads, 8 latent heads). q/k/v and
latent projections are column-sharded per head-group; o_proj/latent_o_proj
are row-sharded, so each core emits a partial output sum and the host adds
the 4 partials per batch.

v2 changes vs baseline (1005us):
  - lk folded on host: lk_eff = w_lq @ w_lk, so the replicated lq
    projection (and its 128 extra matmuls/core) disappears.
  - attention: exp activations run on [128,1024] score PAIRS (halves the
    per-call 352-cycle ScalarE overhead); softmax denominators come from a
    bf16 tree-sum on VectorE + ONE ones-matmul per (head, i-block) instead
    of 12 accumulated ones-matmuls; the M=1 denominator matmuls of 4
    consecutive iterations are col-tiled to partitions 0/32/64/96 of one
    PSUM bank so a single DVE reciprocal (8 cyc/elem, partition-parallel)
    serves 4 iterations; attention output is copied out UNNORMALIZED
    (frees the PSUM bank immediately -> no TensorE stall on the
    reciprocal chain) and rescaled in place afterwards.
  - phase D weights prefetched during attention; B-phase DMAs overlapped
    with compute (8-chunk xt, early wv/wlv issue, k-proj between the two
    wlv half-loads).
"""

import numpy as np
import ml_dtypes

import concourse.bass as bass
import concourse.mybir as mybir
import concourse.tile as tile
from concourse import bacc
from concourse.bass_utils import run_bass_kernel_spmd

BF16 = ml_dtypes.bfloat16

B, S, D = 2, 1024, 4096
H, KVH, HD = 32, 8, 128
GROUPS = H // KVH
LAT, LH = 1024, 32
THETA = 10000.0
SCALE = 1.0 / float(np.sqrt(HD))

NCORES = 8
TP = 4                 # head-group shards
HL = H // TP           # 8 local q heads
KVL = KVH // TP        # 2 local kv heads
LHL = LH // TP         # 8 local latent heads

f32 = mybir.dt.float32
bf16 = mybir.dt.bfloat16

D_T = D // 128         # 32 k-tiles over model dim
S_T = S // 128         # 8 token tiles of 128
IB = 2                 # i-blocks of 512 in attention


def _build_program():
    nc = bacc.Bacc("TRN2", target_bir_lowering=False, debug=False)

    xt_d = nc.dram_tensor("xt", [128, D_T, S], bf16, kind="ExternalInput")
    wq_d = nc.dram_tensor("wq", [HL, 128, D_T, 128], bf16, kind="ExternalInput")
    wk_d = nc.dram_tensor("wk", [KVL, 128, D_T, 128], bf16, kind="ExternalInput")
    wlk_d = nc.dram_tensor("wlk", [LHL, 128, D_T, 128], bf16,
                           kind="ExternalInput")
    wv_d = nc.dram_tensor("wv", [128, D_T, KVL * HD], bf16, kind="ExternalInput")
    wlv_d = nc.dram_tensor("wlv", [2, 128, D_T, 512], bf16,
                           kind="ExternalInput")
    wo_d = nc.dram_tensor("wo", [8, 128, HL, 512], bf16, kind="ExternalInput")
    wlo_d = nc.dram_tensor("wlo", [8, 128, LHL, 512], bf16, kind="ExternalInput")
    cos_d = nc.dram_tensor("cosT", [HD, S], f32, kind="ExternalInput")
    sin_d = nc.dram_tensor("sinTs", [HD, S], f32, kind="ExternalInput")
    mask_d = nc.dram_tensor("maskP2", [128, 2, 1024], bf16, kind="ExternalInput")
    out_d = nc.dram_tensor("out", [S, D], f32, kind="ExternalOutput")

    out_ap = out_d.ap().rearrange("(tt p) d -> p tt d", p=128)

    with tile.TileContext(nc) as tc:
        with tc.tile_pool(name="const", bufs=1) as constp, \
             tc.tile_pool(name="acts", bufs=1) as acts:

            cosT = constp.tile([HD, S], f32, tag="cosT")
            sinTs = constp.tile([HD, S], f32, tag="sinTs")
            maskP2 = constp.tile([128, 2, 1024], bf16, tag="maskP2")
            ones = constp.tile([128, 1], bf16, tag="ones")
            nc.sync.dma_start(cosT[:], cos_d.ap())
            nc.sync.dma_start(sinTs[:], sin_d.ap())
            nc.sync.dma_start(maskP2[:], mask_d.ap())
            nc.vector.memset(ones[:], 1.0)

            # persistent activations (bf16)
            qT = acts.tile([128, HL, S], bf16, tag="qT")
            kT = acts.tile([128, KVL, S], bf16, tag="kT")
            lkT = acts.tile([128, LHL, S], bf16, tag="lkT")
            v_sb = acts.tile([128, S_T, KVL * HD], bf16, tag="v")
            lv_sb = acts.tile([128, S_T, LHL * HD], bf16, tag="lv")

            with tc.tile_pool(name="xt", bufs=1) as xtp, \
                 tc.tile_pool(name="wvlv", bufs=1) as wvp, \
                 tc.tile_pool(name="wstr", bufs=2) as wstr, \
                 tc.tile_pool(name="rope", bufs=2) as ropep, \
                 tc.tile_pool(name="ps_b", bufs=1, space="PSUM") as psb:

                xt = xtp.tile([128, D_T, S], bf16, tag="xt")
                for c in range(8):
                    nc.sync.dma_start(
                        xt[:, bass.ts(c, D_T // 8), :],
                        xt_d.ap()[:, bass.ts(c, D_T // 8), :])

                def rope_to(dst, ps, ib):
                    sl = bass.ts(ib, 512)
                    rt = ropep.tile([128, 512], f32, tag="rt")
                    qc = ropep.tile([128, 512], f32, tag="qc")
                    nc.vector.tensor_tensor(
                        rt[0:64, :], ps[64:128, :], sinTs[0:64, sl],
                        mybir.AluOpType.mult)
                    nc.vector.tensor_tensor(
                        rt[64:128, :], ps[0:64, :], sinTs[64:128, sl],
                        mybir.AluOpType.mult)
                    nc.vector.tensor_tensor(
                        qc[:], ps[:], cosT[:, sl], mybir.AluOpType.mult)
                    nc.vector.tensor_add(dst, qc[:], rt[:])

                def proj_fm(w_dram, n_tiles, dst):
                    # dst[:, nt, :] = rope(w[:, nt-block].T @ xt)
                    for nt in range(n_tiles):
                        wt = wstr.tile([128, D_T, 128], bf16, tag="w_fm")
                        nc.sync.dma_start(wt[:], w_dram.ap()[nt])
                        ps = [psb.tile([128, 512], f32, tag="ps_b1",
                                       bufs=4, name=f"ps_b1_{nt}_{ib}")
                              for ib in range(IB)]
                        for kt in range(D_T):
                            for ib in range(IB):
                                nc.tensor.matmul(
                                    ps[ib][:], wt[:, kt, :],
                                    xt[:, kt, bass.ts(ib, 512)],
                                    start=(kt == 0), stop=(kt == D_T - 1))
                        for ib in range(IB):
                            rope_to(dst[:, nt, bass.ts(ib, 512)], ps[ib][:], ib)

                # ---- q projection first (needs only wq[0] + xt to start) --
                proj_fm(wq_d, HL, qT)

                # ---- B2 weights (prefetch wlv half 0 + wv during q-proj) --
                wv_sb = wvp.tile([128, D_T, KVL * HD], bf16, tag="wv")
                for c in range(4):
                    nc.sync.dma_start(
                        wv_sb[:, bass.ts(c, D_T // 4), :],
                        wv_d.ap()[:, bass.ts(c, D_T // 4), :])

                def b2_half(half, wlv_sb):
                    hs = bass.ts(half, 512)
                    for tt in range(S_T):
                        ps_lv = psb.tile([128, 512], f32, tag="ps_lv",
                                         bufs=2, name=f"ps_lv_{half}_{tt}")
                        if half == 0:
                            ps_v = psb.tile([128, KVL * HD], f32, tag="ps_v",
                                            bufs=2, name=f"ps_v_{tt}")
                        for kt in range(D_T):
                            st = kt == 0
                            sp = kt == D_T - 1
                            lhs = xt[:, kt, bass.ts(tt, 128)]
                            nc.tensor.matmul(ps_lv[:], lhs, wlv_sb[:, kt, :],
                                             start=st, stop=sp)
                            if half == 0:
                                nc.tensor.matmul(ps_v[:], lhs, wv_sb[:, kt, :],
                                                 start=st, stop=sp)
                        nc.any.tensor_copy(lv_sb[:, tt, hs], ps_lv[:])
                        if half == 0:
                            nc.any.tensor_copy(v_sb[:, tt, :], ps_v[:])

                wlv0 = wvp.tile([128, D_T, 512], bf16, tag="wlvh",
                                name="wlvh_0")
                for c in range(2):
                    nc.sync.dma_start(
                        wlv0[:, bass.ts(c, D_T // 2), :],
                        wlv_d.ap()[0][:, bass.ts(c, D_T // 2), :])
                b2_half(0, wlv0)

                # wlv half 1 DMA hides behind k projection
                wlv1 = wvp.tile([128, D_T, 512], bf16, tag="wlvh",
                                name="wlvh_1")
                for c in range(2):
                    nc.sync.dma_start(
                        wlv1[:, bass.ts(c, D_T // 2), :],
                        wlv_d.ap()[1][:, bass.ts(c, D_T // 2), :])
                proj_fm(wk_d, KVL, kT)
                b2_half(1, wlv1)

                # latent k via host-folded weight (same shape as q proj)
                proj_fm(wlk_d, LHL, lkT)

            # ---- phase C: attention (+ phase D weight prefetch) ----------
            with tc.tile_pool(name="attnlat", bufs=1) as alp, \
                 tc.tile_pool(name="wop", bufs=2) as wop, \
                 tc.tile_pool(name="ost", bufs=4) as ost:
                attnT = alp.tile([128, HL, S], bf16, tag="attnT")
                latT = alp.tile([128, LHL, S], bf16, tag="latT")

                def load_wo(np_):
                    wo2 = wop.tile([128, HL, 1024], bf16, tag="wo2",
                                   name=f"wo2_{np_}")
                    wlo2 = wop.tile([128, LHL, 1024], bf16, tag="wlo2",
                                    name=f"wlo2_{np_}")
                    for u in range(2):
                        nc.sync.dma_start(wo2[:, :, bass.ts(u, 512)],
                                          wo_d.ap()[2 * np_ + u])
                        nc.sync.dma_start(wlo2[:, :, bass.ts(u, 512)],
                                          wlo_d.ap()[2 * np_ + u])
                    return wo2, wlo2

                wo_tiles = {0: load_wo(0), 1: load_wo(1)}

                with tc.tile_pool(name="pp", bufs=6) as pp, \
                     tc.tile_pool(name="dn", bufs=1) as dn, \
                     tc.tile_pool(name="ps_c", bufs=1, space="PSUM") as psc:

                    pending = []   # (dst_slice, slot) awaiting normalization
                    flush_q = []   # deferred normalization groups
                    dbatch = None
                    it = 0

                    def emit_flush():
                        # normalize a completed 4-iteration group, staged
                        # across two later iterations so no single VectorE
                        # block (esp. the 8-cyc/elem reciprocal) sits in
                        # front of the next iteration's mask/tree work
                        st = flush_q[0]
                        grp_dbatch, grp, stage, rec, recbs = st
                        if stage == 0:
                            rec = dn.tile([128, 512], f32, tag="rec", bufs=2)
                            nc.vector.reciprocal(rec[0:97, 0:256],
                                                 grp_dbatch[0:97, 0:256])
                            st[2] = 1
                            st[3] = rec
                            return
                        if stage == 1:
                            nc.vector.reciprocal(rec[0:97, 256:512],
                                                 grp_dbatch[0:97, 256:512])
                            # bf16 copy of rec: keeps the final multiplies
                            # pure-bf16 (mixed bf16xf32 TT runs 1x on DVE)
                            recbf = dn.tile([128, 512], bf16, tag="recbf",
                                            bufs=2)
                            nc.vector.tensor_copy(recbf[0:97, :],
                                                  rec[0:97, :])
                            for _, pslot in grp:
                                # partition_broadcast only reads partition 0
                                # on HW: stage the row there via SBUF DMA
                                if pslot == 0:
                                    src = recbf[0:1, :]
                                else:
                                    r0 = dn.tile([1, 512], bf16, tag="r0",
                                                 bufs=2)
                                    nc.sync.dma_start(
                                        r0[0:1, :],
                                        recbf[32 * pslot:32 * pslot + 1, :])
                                    src = r0[0:1, :]
                                recb = dn.tile([128, 512], bf16, tag="recb",
                                               bufs=4)
                                nc.gpsimd.partition_broadcast(recb[:], src)
                                recbs.append(recb)
                            st[2] = 2
                            return
                        for (pdsl, _), recb in zip(grp, recbs):
                            nc.vector.tensor_tensor(
                                pdsl, pdsl, recb[:], mybir.AluOpType.mult)
                        flush_q.pop(0)

                    for vh in range(HL + LHL):
                        if vh < HL:
                            h = vh
                            ksrc = kT[:, h // GROUPS, :]
                            dst = attnT
                        else:
                            h = vh - HL
                            ksrc = lkT[:, h, :]
                            dst = latT
                        qsrc = qT[:, h, :]

                        for ib in range(IB):
                            if flush_q:
                                emit_flush()
                            npair = 2 * (ib + 1)
                            isl = bass.ts(ib, 512)
                            pts = []
                            for pb in range(npair):
                                ps_pair = psc.tile([128, 1024], f32,
                                                   tag="ps_pair", bufs=2)
                                for u in range(2):
                                    nc.tensor.matmul(
                                        ps_pair[:, bass.ts(u, 512)],
                                        ksrc[:, bass.ts(2 * pb + u, 128)],
                                        qsrc[:, isl], start=True, stop=True)
                                pt = pp.tile([128, 1024], bf16, tag="pt")
                                nc.scalar.activation(
                                    pt[:], ps_pair[:],
                                    mybir.ActivationFunctionType.Exp,
                                    scale=SCALE)
                                # diagonal pairs get the causal 0/1 mask
                                r = 2 * pb - 4 * ib
                                if r >= 0:
                                    nc.vector.tensor_tensor(
                                        pt[:], pt[:], maskP2[:, r // 2, :],
                                        mybir.AluOpType.mult)
                                pts.append(pt)

                            ps_o = psc.tile([128, 512], f32, tag="ps_o",
                                            bufs=2)
                            for jb in range(2 * npair):
                                if vh < HL:
                                    vsl = v_sb[:, jb, bass.ts(h // GROUPS, HD)]
                                else:
                                    vsl = lv_sb[:, jb, bass.ts(h, HD)]
                                nc.tensor.matmul(
                                    ps_o[:], vsl,
                                    pts[jb // 2][:, bass.ts(jb % 2, 512)],
                                    start=(jb == 0), stop=(jb == 2 * npair - 1))

                            # denominator: bf16 tree-sum then one M=1 matmul
                            # col-tiled into partition 32*slot of dbatch
                            psums = []
                            for pt in pts:
                                sp_ = dn.tile([128, 512], bf16, tag="tadd",
                                              bufs=6)
                                nc.vector.tensor_add(sp_[:], pt[:, 0:512],
                                                     pt[:, 512:1024])
                                psums.append(sp_)

                            # accumulate pair-sums on TensorE (slack in C)
                            # instead of more VectorE tree levels
                            slot = it % 4
                            if slot == 0:
                                dbatch = psc.tile([128, 512], f32,
                                                  tag="dbatch", bufs=2)
                                nc.vector.memset(dbatch[:], 1.0)
                            for pb, sp_ in enumerate(psums):
                                nc.tensor.matmul(
                                    dbatch[32 * slot:32 * slot + 1, :],
                                    ones[:, 0:1], sp_[:],
                                    start=(pb == 0),
                                    stop=(pb == len(psums) - 1),
                                    tile_position=(0, 32 * slot))

                            # unnormalized copy-out frees the PSUM bank now
                            dsl = dst[:, h, isl]
                            nc.any.tensor_copy(dsl, ps_o[:])
                            pending.append((dsl, slot))

                            if slot == 3:
                                flush_q.append([dbatch, pending, 0, None, []])
                                pending = []
                            it += 1

                    while flush_q:
                        emit_flush()

                # ---- phase D: output projections (row-sharded partials) --
                with tc.tile_pool(name="ps_f", bufs=4, space="PSUM") as psf:
                    for np_ in range(4):
                        wo2, wlo2 = wo_tiles.pop(np_)
                        if np_ + 2 < 4:
                            wo_tiles[np_ + 2] = load_wo(np_ + 2)
                        for tt in range(S_T):
                            ps0 = psf.tile([128, 512], f32, tag="ps_f")
                            ps1 = psf.tile([128, 512], f32, tag="ps_f")
                            for h in range(HL):
                                lhs = attnT[:, h, bass.ts(tt, 128)]
                                nc.tensor.matmul(ps0[:], lhs,
                                                 wo2[:, h, 0:512],
                                                 start=(h == 0), stop=False)
                                nc.tensor.matmul(ps1[:], lhs,
                                                 wo2[:, h, 512:1024],
                                                 start=(h == 0), stop=False)
                            for h in range(LHL):
                                lhs = latT[:, h, bass.ts(tt, 128)]
                                nc.tensor.matmul(ps0[:], lhs,
                                                 wlo2[:, h, 0:512],
                                                 start=False, stop=(h == LHL - 1))
                                nc.tensor.matmul(ps1[:], lhs,
                                                 wlo2[:, h, 512:1024],
                                                 start=False, stop=(h == LHL - 1))
                            for u, ps in enumerate((ps0, ps1)):
                                ot = ost.tile([128, 512], f32, tag="ot")
                                nc.any.tensor_copy(ot[:], ps[:])
                                nc.sync.dma_start(
                                    out_ap[:, tt, bass.ds(
                                        (2 * np_ + u) * 512, 512)],
                                    ot[:])

    nc.compile()
    return nc


_NC = None


def _get_program():
    global _NC
    if _NC is None:
        _NC = _build_program()
    return _NC


def _rope_tables():
    inv_freq = 1.0 / (THETA ** (np.arange(0, HD, 2, dtype=np.float32) / HD))
    t = np.arange(S, dtype=np.float32)
    freqs = np.outer(t, inv_freq)                       # [S, 64]
    emb = np.concatenate([freqs, freqs], axis=-1)       # [S, HD]
    cosT = np.cos(emb).T.astype(np.float32).copy()      # [HD, S]
    sinT = np.sin(emb).T.astype(np.float32)
    sinTs = np.concatenate([-sinT[:HD // 2], sinT[HD // 2:]], 0).astype(
        np.float32).copy()
    return cosT, sinTs


def _mask_patterns():
    # maskP2[p, t, u*512 + i] = 1.0 iff ((2t+u)*128 + p) <= i, i in [0,512)
    p = np.arange(128)[:, None, None]
    ju = np.arange(4)[None, :, None]        # jb index within diagonal block
    i = np.arange(512)[None, None, :]
    m = ((ju * 128 + p) <= i).astype(BF16)  # [128, 4, 512]
    return np.ascontiguousarray(
        m.reshape(128, 2, 2, 512).reshape(128, 2, 1024))


def _tile_w_fm(w, n_tiles, kt):
    # [K, n_tiles*128] -> [n_tiles, 128(p of K), kt, 128]
    K, N = w.shape
    assert K == kt * 128 and N == n_tiles * 128
    return np.ascontiguousarray(
        w.reshape(kt, 128, n_tiles, 128).transpose(2, 1, 0, 3)).astype(BF16)


def _tile_w_tm(w, kt):
    # [K, N] -> [128(p of K), kt, N]
    K, N = w.shape
    assert K == kt * 128
    return np.ascontiguousarray(
        w.reshape(kt, 128, N).transpose(1, 0, 2)).astype(BF16)


def _tile_w_lv(w):
    # [D, 1024] -> [2(half), 128(p of K), D_T, 512]
    t = _tile_w_tm(w, D_T)                  # [128, D_T, 1024]
    return np.ascontiguousarray(
        t.reshape(128, D_T, 2, 512).transpose(2, 0, 1, 3))


def _tile_w_out(w):
    # [1024, D] -> [8(nb), 128(p of rows), 8(h), 512]
    return np.ascontiguousarray(
        w.reshape(8, 128, D // 512, 512).transpose(2, 1, 0, 3)).astype(BF16)


def _make_in_maps(hidden_states, w_q, w_k, w_v, w_o, w_lq, w_lk, w_lv, w_lo):
    cosT, sinTs = _rope_tables()
    maskP2 = _mask_patterns()

    # host fold: lk_eff = w_lq @ w_lk  [D, LH*HD]
    lk_eff = np.asarray(w_lq, dtype=np.float32) @ np.asarray(
        w_lk, dtype=np.float32)

    g_maps = []
    for g in range(TP):
        qs = slice(g * HL * HD, (g + 1) * HL * HD)
        kvs = slice(g * KVL * HD, (g + 1) * KVL * HD)
        ls = slice(g * LHL * HD, (g + 1) * LHL * HD)
        g_maps.append({
            "wq": _tile_w_fm(np.asarray(w_q)[:, qs], HL, D_T),
            "wk": _tile_w_fm(np.asarray(w_k)[:, kvs], KVL, D_T),
            "wlk": _tile_w_fm(lk_eff[:, ls], LHL, D_T),
            "wv": _tile_w_tm(np.asarray(w_v)[:, kvs], D_T),
            "wlv": _tile_w_lv(np.asarray(w_lv)[:, ls]),
            "wo": _tile_w_out(np.asarray(w_o)[qs, :]),
            "wlo": _tile_w_out(np.asarray(w_lo)[ls, :]),
            "cosT": cosT,
            "sinTs": sinTs,
            "maskP2": maskP2,
        })

    xts = []
    for b in range(B):
        x = np.asarray(hidden_states[b], dtype=np.float32)       # [S, D]
        xts.append(np.ascontiguousarray(
            x.T.reshape(D_T, 128, S).transpose(1, 0, 2)).astype(BF16))

    in_maps = []
    for c in range(NCORES):
        b, g = divmod(c, TP)
        m = dict(g_maps[g])
        m["xt"] = xts[b]
        in_maps.append(m)
    return in_maps


def kernel(hidden_states, w_q, w_k, w_v, w_o, w_lq, w_lk, w_lv, w_lo):
    nc = _get_program()
    in_maps = _make_in_maps(hidden_states, w_q, w_k, w_v, w_o,
                            w_lq, w_lk, w_lv, w_lo)
    res = run_bass_kernel_spmd(nc, in_maps, list(range(NCORES))).results

    out = np.zeros((B, S, D), dtype=np.float32)
    for c in range(NCORES):
        b = c // TP
        out[b] += res[c]["out"]
    return out
